# revision 49
# baseline (speedup 1.0000x reference)
"""
Trainium2 Bass kernel for nn_Local_Attention (segment-softmax attention over
atoms grouped into tokens).

Algorithm notes (reference semantics):
  q = (q_x @ Wq + bq) / sqrt(C)            [N, H*C]
  k = kv_x @ Wk ; v = kv_x @ Wv            [N, H*C]
  s[i,h] = sum_c q[i,h,c] k[i,h,c] / sqrt(C)
  alpha  = softmax of s over atoms sharing a token (segment softmax)
  out[t] = sum_{i in t} alpha[i] * v[i]    (only rows t < NUM_TOKENS nonzero)
  result = (out * sigmoid(q_x @ Wg)) @ Wo + bo

Key simplifications used here:
  * Scores are tiny (|s| < ~0.1), so the segment-max subtraction is skipped:
    alpha = e / segsum(e), e = exp(s). Numerator and denominator are both
    segment *sums*, and the division happens at token level:
    out[t] = segsum(e*v)[t] / segsum(e)[t].
  * Rows >= NUM_TOKENS of the result equal bo (segment sum there is zero), so
    only the first NUM_TOKENS rows are computed on device.
  * Segment sums are computed as one-hot matmuls: the host sorts atoms by
    token, packs 128 tokens per "group" (LPT-balanced), pads each group's
    atom list to a fixed tile capacity, and each 128-atom tile contributes
    via a [atom, token-slot] one-hot built on host from per-atom slot ids.

Sharding: 128 groups of 128 tokens each; 16 groups per core on 8 cores.
Projection weights are replicated.

v2.3 structure (changes vs the v1 baseline):
  * q_x / Wq / Wg / gate-token-inputs in fp8e4m3 (scores+gate arguments only;
    kv_x stays bf16). Wq,Wg are pre-scaled by 32 on the host to stay in fp8
    normal range; the 1/32 comes back via the score-reduction matrix `ind`
    resp. the sigmoid's scale argument.
  * Input DMAs split across the two hardware queues (sync + scalar) and
    issued as 2-group chunks so each DMA row is 4-8KB contiguous (the
    per-queue packet rate limits throughput for small rows).
  * 4-tile batches (512-wide PSUM tiles); scores accumulate in spare columns
    of the double-buffered per-group [num|den|s] PSUM bank. All matmuls into
    that bank form ONE accumulation group (a second start=True would clear
    the whole bank's has_written state).
  * The per-tile segment-sum matmuls are software-pipelined one batch behind
    the rest of the chain, so the PE's in-order queue never stalls waiting
    for the DVE's e*v multiply (stalls re-throttle the PE clock via HAM).
  * Token-level normalize+gate is one DVE broadcast mult (gate*recip) plus
    one DVE scalar_tensor_tensor (num * gateR).
  * Output projection (transpose + Wo matmul) runs inline every 4 groups,
    writing a contiguous [128, 512] block per 4 groups.
"""

import math
import os
import sys

import numpy as np

sys.path.insert(0, "/opt/trn_rl_repo")

import ml_dtypes

BF16 = ml_dtypes.bfloat16
FP8 = ml_dtypes.float8_e4m3

N = 262144
C_Q = 128
C_KV = 128
H = 4
C = 32
HC = H * C  # 128
NUM_TOKENS = 16384
NCORES = 8
GROUPS = 128          # token groups overall
TPG = 128             # tokens per group
GPC = GROUPS // NCORES  # groups per core = 16
INV_SQRT_C = 1.0 / math.sqrt(C)
WQ_PRESCALE = 32.0    # host multiplies Wq/Wg by this; undone via ind / scale

_BUILD_CACHE = {}
LAST_RESULTS = None  # stash of the last BassKernelResults for test harness


def _host_shard(atom_to_token_idx):
    """Assign tokens to 128 LPT-balanced groups of 128 tokens, sort atoms by
    (group, token), and compute the padded layout.

    Returns dict with permutation, destination indices, per-atom slot ids,
    token grid, and cap_tiles."""
    import heapq

    idx = np.asarray(atom_to_token_idx).astype(np.int64)
    counts = np.bincount(idx, minlength=NUM_TOKENS)

    # LPT: tokens sorted by size desc, each to the least-loaded group with
    # room; then local swaps to push the max group load to the mean
    order_tok = np.argsort(-counts, kind="stable")
    loads = np.zeros(GROUPS, np.int64)
    ntok = np.zeros(GROUPS, np.int64)
    grp_of_tok = np.empty(NUM_TOKENS, np.int64)
    heap = [(0, g) for g in range(GROUPS)]
    heapq.heapify(heap)
    for t in order_tok:
        while True:
            _, g = heapq.heappop(heap)
            if ntok[g] < TPG:
                break
        grp_of_tok[t] = g
        loads[g] += counts[t]
        ntok[g] += 1
        if ntok[g] < TPG:
            heapq.heappush(heap, (loads[g], g))
    target = N // GROUPS  # 2048
    tok_by_grp = [list(np.where(grp_of_tok == g)[0]) for g in range(GROUPS)]
    for _ in range(2000):
        mx = int(loads.argmax())
        if loads[mx] <= target:
            break
        mn = int(loads.argmin())
        need = loads[mx] - target
        best = None
        for a in tok_by_grp[mx]:
            for b in tok_by_grp[mn]:
                dlt = counts[a] - counts[b]
                if dlt > 0 and loads[mn] + dlt <= target:
                    sc = abs(dlt - need)
                    if best is None or sc < best[0]:
                        best = (sc, a, b)
        if best is None:
            break
        _, a, b = best
        tok_by_grp[mx].remove(a)
        tok_by_grp[mn].remove(b)
        tok_by_grp[mx].append(b)
        tok_by_grp[mn].append(a)
        loads[mx] += counts[b] - counts[a]
        loads[mn] += counts[a] - counts[b]
        grp_of_tok[a], grp_of_tok[b] = mn, mx
    slot_of_tok = np.empty(NUM_TOKENS, np.int64)
    tok_grid = np.empty((GROUPS, TPG), np.int64)
    for g in range(GROUPS):
        toks = tok_by_grp[g]
        tok_grid[g] = toks
        slot_of_tok[toks] = np.arange(TPG)
    loads = counts[tok_grid].sum(axis=1)  # atoms per group
    cap_tiles = max(1, int(math.ceil(loads.max() / 128.0)))
    cap_atoms = cap_tiles * 128

    # atoms sorted by (group, token id)
    key = grp_of_tok[idx] * NUM_TOKENS + idx
    perm = np.argsort(key, kind="stable")
    gidx = grp_of_tok[idx[perm]]           # nondecreasing group per atom
    group_start = np.searchsorted(gidx, np.arange(GROUPS))
    rank = np.arange(N) - group_start[gidx]
    dest = gidx * cap_atoms + rank         # position in padded atom array
    slots = slot_of_tok[idx[perm]]         # token slot of each (permuted) atom

    return dict(
        perm=perm,
        dest=dest,
        slots=slots,
        tok_grid=tok_grid,
        counts=counts,
        cap_tiles=cap_tiles,
        cap_atoms=cap_atoms,
    )


def _batches_for(cap_tiles):
    """Batches of up to 4 tiles; avoid a trailing batch of 1."""
    batches = []
    b0 = 0
    while b0 < cap_tiles:
        rem = cap_tiles - b0
        if rem == 5:
            B = 3
        else:
            B = min(4, rem)
        batches.append((b0, B))
        b0 += B
    return batches


def _build_nc(cap_tiles, has_bq=False, debug_dumps=False):
    """Build + schedule the SPMD Bass program for a given per-group tile
    capacity. Cached per cap_tiles."""
    key = (cap_tiles, has_bq, debug_dumps)
    if key in _BUILD_CACHE:
        return _BUILD_CACHE[key]

    import concourse.bass as bass
    import concourse.tile as tile
    from concourse import bacc, mybir

    dt = mybir.dt
    AOT = mybir.AluOpType
    AFT = mybir.ActivationFunctionType

    cap_atoms = cap_tiles * 128
    atoms_pc = GPC * cap_atoms         # padded atoms per core
    GS_COLS = 132 + 4 * cap_tiles      # group bank: [num 128 | den 4 | s ...]
    assert GS_COLS <= 512

    nc = bacc.Bacc(
        "TRN2", target_bir_lowering=False, debug=False, num_devices=NCORES
    )

    qxT_d = nc.dram_tensor("qxT", [128, atoms_pc], dt.float8e4, kind="ExternalInput")
    kvT_d = nc.dram_tensor("kvT", [128, atoms_pc], dt.bfloat16, kind="ExternalInput")
    # bf16 constants: cols = [wk 128 | wv 128 | wo 128 | ind 4 | ident 128]
    CONST_COLS = 3 * 128 + H + 128
    const_d = nc.dram_tensor("consts", [128, CONST_COLS], dt.bfloat16, kind="ExternalInput")
    # fp8 constants: cols = [wq8 128 | wg8 128 | qxoT8 2048]
    F8_COLS = 2 * 128 + GPC * TPG
    f8c_d = nc.dram_tensor("f8c", [128, F8_COLS], dt.float8e4, kind="ExternalInput")
    a_d = nc.dram_tensor("aT", [128, atoms_pc], dt.float8e4, kind="ExternalInput")
    bq_d = nc.dram_tensor("bqv", [128, 1], dt.float32, kind="ExternalInput")
    # out: phase-major [(GPC//4)*128, 4*C_Q]; host untangles
    out_d = nc.dram_tensor("out", [(GPC // 4) * 128, 4 * C_Q], dt.float32, kind="ExternalOutput")
    dbg = {}
    if debug_dumps:
        for nm, shp, dty in [
            ("dbg_gs", [128, GS_COLS], dt.float32),
            ("dbg_r32", [128, H], dt.float32),
            ("dbg_gateR", [128, TPG], dt.bfloat16),
            ("dbg_y", [128, TPG], dt.bfloat16),
            ("dbg_we", [128, 4 * 132], dt.bfloat16),
            ("dbg_q2", [128, 512], dt.bfloat16),
            ("dbg_qk", [128, 512], dt.bfloat16),
        ]:
            dbg[nm] = nc.dram_tensor(nm, shp, dty, kind="ExternalOutput")

    batches = _batches_for(cap_tiles)

    with tile.TileContext(nc) as tc:
        with (
            tc.tile_pool(name="const", bufs=1) as cpool,
            tc.tile_pool(name="inp", bufs=5) as inp,
            tc.tile_pool(name="kvp", bufs=GPC // 2) as kvpool,
            tc.tile_pool(name="sb", bufs=3) as sb,
            tc.tile_pool(name="outp", bufs=2) as outp,
            tc.tile_pool(name="pq", bufs=2, space=bass.MemorySpace.PSUM) as pq,
            tc.tile_pool(name="pk", bufs=2, space=bass.MemorySpace.PSUM) as pk,
            tc.tile_pool(name="pv", bufs=2, space=bass.MemorySpace.PSUM) as pv,
            tc.tile_pool(name="pgs", bufs=2, space=bass.MemorySpace.PSUM) as pgs,
        ):
            f8c_sb = cpool.tile([128, F8_COLS], dt.float8e4)
            nc.sync.dma_start(f8c_sb[:], f8c_d[:])
            const_sb = cpool.tile([128, CONST_COLS], dt.bfloat16)
            nc.sync.dma_start(const_sb[:], const_d[:])
            wq8_sb = f8c_sb[:, 0:128]
            wg8_sb = f8c_sb[:, 128:256]
            qxo8_sb = f8c_sb[:, 256 : 256 + GPC * TPG]
            wk_sb = const_sb[:, 0:128]
            wv_sb = const_sb[:, 128:256]
            wo_sb = const_sb[:, 256:384]
            ind_sb = const_sb[:, 384 : 384 + H]
            ident_sb = const_sb[:, 388:516]
            bq_sb = cpool.tile([128, 1], dt.float32)
            if has_bq:
                nc.sync.dma_start(bq_sb[:], bq_d[:])

            gate_all = cpool.tile([128, GPC * TPG], dt.bfloat16)
            y_all = cpool.tile([128, GPC * TPG], dt.bfloat16)

            # gate pre-pass for all groups up front, 4 groups per PSUM bank
            # (also warms the PE while the first input DMAs land)
            for g4 in range(0, GPC, 4):
                g_ps = pq.tile([128, 4 * HC], dt.float32, tag="qp")
                for j in range(4):
                    g = g4 + j
                    nc.tensor.matmul(
                        g_ps[:, j * HC : (j + 1) * HC],
                        qxo8_sb[:, g * TPG : (g + 1) * TPG],
                        wg8_sb,
                        start=True, stop=True,
                    )
                nc.scalar.activation(
                    gate_all[:, g4 * TPG : (g4 + 4) * TPG], g_ps[:],
                    AFT.Sigmoid, scale=1.0 / WQ_PRESCALE,
                )

            # deferred per-tile segment-sum matmuls: one batch behind the
            # main chain so the PE queue never blocks on the DVE e*v mult
            pending = []
            bidx = [0]  # global batch counter (for DVE->gpsimd offload)

            def emit_tail(g, gs):
                # y = num * (gate / den) into y_all
                if debug_dumps and g == 0:
                    gs_cp = sb.tile([128, GS_COLS], dt.float32, tag="gscp")
                    nc.vector.tensor_copy(gs_cp[:], gs[:])
                    nc.sync.dma_start(dbg["dbg_gs"][:], gs_cp[:])
                r32 = sb.tile([128, H], dt.float32, tag="r32")
                nc.vector.reciprocal(r32[:], gs[:, 128:132])
                gateR = sb.tile([128, TPG], dt.bfloat16, tag="gr")
                nc.gpsimd.tensor_tensor(
                    gateR[:].rearrange("p (h c) -> p h c", h=H, c=C),
                    gate_all[:, g * TPG : (g + 1) * TPG].rearrange(
                        "p (h c) -> p h c", h=H, c=C
                    ),
                    r32[:].unsqueeze(-1).broadcast_to((128, H, C)),
                    AOT.mult,
                )
                nc.vector.scalar_tensor_tensor(
                    y_all[:, g * TPG : (g + 1) * TPG],
                    gs[:, 0:128],
                    1.0,
                    gateR[:],
                    AOT.mult,
                    AOT.mult,
                )
                if debug_dumps and g == 0:
                    nc.sync.dma_start(dbg["dbg_r32"][:], r32[:])
                    nc.sync.dma_start(dbg["dbg_gateR"][:], gateR[:])
                    nc.sync.dma_start(dbg["dbg_y"][:], y_all[:, 0:TPG])
                if g % 4 == 3:
                    emit_phase2(g - 3)

            def emit_phase2(g4, j0=0, nj=4):
                # transpose + output projection for groups g4+j0..g4+j0+nj-1
                W = nj * 128
                yT_ps = pk.tile([128, W], dt.bfloat16, tag="kp")
                for j in range(nj):
                    gg = g4 + j0 + j
                    nc.tensor.transpose(
                        yT_ps[:, j * 128 : (j + 1) * 128],
                        y_all[:, gg * TPG : (gg + 1) * TPG],
                        ident_sb,
                    )
                yT16 = sb.tile([128, W], dt.bfloat16, tag="yT")
                nc.scalar.activation(yT16[:], yT_ps[:], AFT.Copy)
                f_ps = pv.tile([128, W], dt.float32, tag="vp")
                for j in range(nj):
                    nc.tensor.matmul(
                        f_ps[:, j * 128 : (j + 1) * 128],
                        yT16[:, j * 128 : (j + 1) * 128],
                        wo_sb,
                        start=True, stop=True,
                    )
                o32 = outp.tile([128, W], dt.float32, tag="o")
                nc.scalar.activation(o32[:], f_ps[:], AFT.Copy)
                ph = g4 // 4
                nc.sync.dma_start(
                    out_d[ph * 128 : (ph + 1) * 128, j0 * 128 : (j0 + nj) * 128],
                    o32[:],
                )

            def flush_pending(keep=0):
                while len(pending) > keep:
                    p = pending.pop(0)
                    for b in range(p["B"]):
                        t = p["b0"] + b
                        nc.tensor.matmul(
                            p["gs"][:, 0:132],
                            p["a16"][:, 128 * t : 128 * (t + 1)],
                            p["we"][:, 132 * b : 132 * (b + 1)],
                            start=False, stop=(t == cap_tiles - 1),
                            skip_group_check=True,
                        )
                    if p["last"]:
                        emit_tail(p["g"], p["gs"])

            for gp in range(0, GPC, 2):
                ca2 = 2 * cap_atoms
                qx_p = inp.tile([128, ca2], dt.float8e4, tag="qx")
                nc.sync.dma_start(qx_p[:], qxT_d[:, gp * cap_atoms : gp * cap_atoms + ca2])
                kv_p = kvpool.tile([128, ca2], dt.bfloat16, tag="kv")
                nc.scalar.dma_start(kv_p[:], kvT_d[:, gp * cap_atoms : gp * cap_atoms + ca2])
                a_p = inp.tile([128, ca2], dt.float8e4, tag="a")
                nc.sync.dma_start(a_p[:], a_d[:, gp * cap_atoms : gp * cap_atoms + ca2])

                for gi in range(2):
                    g = gp + gi
                    qx_g = qx_p[:, gi * cap_atoms : (gi + 1) * cap_atoms]
                    kv_g = kv_p[:, gi * cap_atoms : (gi + 1) * cap_atoms]
                    a16 = a_p[:, gi * cap_atoms : (gi + 1) * cap_atoms]

                    # group bank: cols 0:128 num accum, 128:132 den accum,
                    # 132+4t : s of tile t. ONE accumulation group: only the
                    # first write (s of tile 0) carries start=True; a second
                    # start would clear the whole bank's has_written bits.
                    gs = pgs.tile([128, GS_COLS], dt.float32, tag="gs")

                    for (b0, B) in batches:
                        A = B * 128
                        off = b0 * 128
                        # feature-major q, k for the score chain
                        q_ps = pq.tile([128, A], dt.float32, tag="qp")
                        nc.tensor.matmul(
                            q_ps[:], wq8_sb, qx_g[:, off : off + A],
                            start=True, stop=True,
                        )
                        k_ps = pk.tile([128, A], dt.float32, tag="kp")
                        nc.tensor.matmul(
                            k_ps[:], wk_sb, kv_g[:, off : off + A],
                            start=True, stop=True,
                        )
                        # q2 = q (+ bq) on ACT, PSUM -> SBUF; then
                        # qk = q2 * k on DVE (only one PSUM operand allowed)
                        q2 = sb.tile([128, A], dt.bfloat16, tag="q2")
                        if has_bq:
                            nc.scalar.activation(
                                q2[:], q_ps[:], AFT.Identity, bias=bq_sb[:]
                            )
                        else:
                            nc.scalar.activation(q2[:], q_ps[:], AFT.Copy)
                        qk = sb.tile([128, A], dt.bfloat16, tag="qk")
                        if bidx[0] % 4 == 2:
                            # ACT drains k too; qk then runs in the DVE's
                            # 2x packed bf16 mode (both operands SBUF)
                            k2 = sb.tile([128, A], dt.bfloat16, tag="k2")
                            nc.scalar.activation(k2[:], k_ps[:], AFT.Copy)
                            nc.vector.tensor_tensor(qk[:], q2[:], k2[:], AOT.mult)
                        else:
                            nc.vector.tensor_tensor(qk[:], q2[:], k_ps[:], AOT.mult)
                        # atom-major v for this batch (one PSUM bank)
                        v_ps = pv.tile([128, A], dt.float32, tag="vp")
                        for b in range(B):
                            nc.tensor.matmul(
                                v_ps[:, 128 * b : 128 * (b + 1)],
                                kv_g[:, off + 128 * b : off + 128 * (b + 1)],
                                wv_sb,
                                start=True, stop=True,
                            )
                        # s[atom, h] per tile via PE reduction over hc
                        # partitions (ind carries the 1/WQ_PRESCALE)
                        for b in range(B):
                            t = b0 + b
                            nc.tensor.matmul(
                                gs[:, 132 + 4 * t : 136 + 4 * t],
                                qk[:, 128 * b : 128 * (b + 1)],
                                ind_sb,
                                start=(t == 0), stop=False,
                                skip_group_check=True,
                            )
                        # fused rhs tile: per tile 132 cols = [w (128) | e(4)]
                        # e = exp(s) written straight into the e columns
                        we = sb.tile([128, B * 132], dt.bfloat16, tag="we")
                        we3 = we[:].rearrange("p (b f) -> p b f", b=B, f=132)
                        nc.scalar.activation(
                            we3[:, :, 128:132],
                            gs[:, 132 + 4 * b0 : 132 + 4 * (b0 + B)].rearrange(
                                "p (b h) -> p b h", b=B, h=H
                            ),
                            AFT.Exp,
                        )
                        w_view = we3[:, :, 0:128].rearrange(
                            "p b (h c) -> p b h c", h=H, c=C
                        )
                        e_view = (
                            we3[:, :, 128:132]
                            .unsqueeze(-1)
                            .broadcast_to((128, B, H, C))
                        )
                        v_view = v_ps[:, 0 : 128 * B].rearrange(
                            "p (b h c) -> p b h c", b=B, h=H, c=C
                        )
                        nc.vector.tensor_tensor(w_view, v_view, e_view, AOT.mult)
                        bidx[0] += 1
                        if debug_dumps and g == 0 and b0 == 0:
                            nc.sync.dma_start(dbg["dbg_q2"][:], q2[:])
                            nc.sync.dma_start(dbg["dbg_qk"][:], qk[:])
                            nc.sync.dma_start(dbg["dbg_we"][:], we[:])
                        flush_pending()
                        pending.append(dict(
                            gs=gs, a16=a16, we=we, b0=b0, B=B, g=g,
                            last=(b0 + B == cap_tiles),
                        ))

            flush_pending()

    nc.compile()
    _BUILD_CACHE[key] = nc
    return nc


def _install_ntff_shim():
    """The agent image's `antenv` lacks `axon_hooks`; recreate it and install
    the ctypes NTFF profile hook the way trn_agent_boot would."""
    import types

    import antenv

    if "antenv.axon_hooks" in sys.modules:
        return
    mod = types.ModuleType("antenv.axon_hooks")
    holder = [None]
    mod.set_axon_ntff_profile_hook = lambda h: holder.__setitem__(0, h)
    mod.get_axon_ntff_profile_hook = lambda: holder[0]
    sys.modules["antenv.axon_hooks"] = mod
    antenv.axon_hooks = mod
    try:
        sys.path.insert(0, "/root/.axon_site")
        from trn_agent_boot.trn_boot import _ntff_profile_via_ctypes

        hook = _ntff_profile_via_ctypes("/opt/axon/libaxon_pjrt.so")
        mod.set_axon_ntff_profile_hook(hook)
    except Exception as e:  # degrade to no tracing
        print(f"ntff shim install failed: {e}")


def kernel(q_x, kv_x, atom_to_token_idx, Wq, bq, Wk, Wv, Wg, Wo, bo):
    global LAST_RESULTS
    from concourse.bass_utils import run_bass_kernel_spmd

    q_x = np.asarray(q_x, np.float32)
    kv_x = np.asarray(kv_x, np.float32)
    Wq = np.asarray(Wq, np.float32)
    bq = np.asarray(bq, np.float32)
    Wk = np.asarray(Wk, np.float32)
    Wv = np.asarray(Wv, np.float32)
    Wg = np.asarray(Wg, np.float32)
    Wo = np.asarray(Wo, np.float32)
    bo = np.asarray(bo, np.float32)

    sh = _host_shard(atom_to_token_idx)
    cap_tiles = sh["cap_tiles"]
    cap_atoms = sh["cap_atoms"]
    perm, dest, slots = sh["perm"], sh["dest"], sh["slots"]
    tok_grid = sh["tok_grid"]

    # padded, permuted inputs (q_x in fp8 for the score path)
    tot = GROUPS * cap_atoms
    Xq = np.zeros((tot, 128), FP8)
    Xq[dest] = q_x[perm].astype(FP8)
    Xkv = np.zeros((tot, 128), BF16)
    Xkv[dest] = kv_x[perm].astype(BF16)
    Afull = np.zeros((tot, TPG), FP8)
    Afull[dest, slots] = 1

    wq8_h = (Wq * (INV_SQRT_C * WQ_PRESCALE)).astype(FP8)
    wg8_h = (Wg * WQ_PRESCALE).astype(FP8)
    wk_h = (Wk * INV_SQRT_C).astype(BF16)
    wv_h = Wv.astype(BF16)
    wo_h = Wo.astype(BF16)
    bq_h = (bq * (INV_SQRT_C * WQ_PRESCALE)).astype(np.float32).reshape(128, 1)
    ind_h = np.zeros((HC, H), BF16)
    for h in range(H):
        ind_h[h * C : (h + 1) * C, h] = 1.0 / WQ_PRESCALE
    ident_h = np.eye(128, dtype=BF16)

    apc = GPC * cap_atoms
    in_maps = []
    for c in range(NCORES):
        rows = slice(c * apc, (c + 1) * apc)
        qxT = np.ascontiguousarray(Xq[rows].T)
        kvT = np.ascontiguousarray(Xkv[rows].T)
        aT = np.ascontiguousarray(
            Afull[rows]
            .reshape(GPC * cap_tiles, 128, TPG)
            .transpose(1, 0, 2)
            .reshape(128, apc)
        )
        tok_core = tok_grid[c * GPC : (c + 1) * GPC].reshape(GPC * TPG)
        qxo8 = np.ascontiguousarray(q_x[tok_core].T.astype(FP8))
        const_h = np.concatenate([wk_h, wv_h, wo_h, ind_h, ident_h], axis=1)
        f8c_h = np.concatenate([wq8_h, wg8_h, qxo8], axis=1)
        in_maps.append(
            dict(qxT=qxT, kvT=kvT, aT=aT, consts=const_h, f8c=f8c_h, bqv=bq_h)
        )

    nc = _build_nc(
        cap_tiles,
        has_bq=bool(np.any(bq != 0)),
        debug_dumps=os.environ.get("KERNEL_DEBUG_DUMPS", "0") == "1",
    )
    trace = os.environ.get("KERNEL_TRACE", "0") == "1"
    if trace:
        _install_ntff_shim()
    res = run_bass_kernel_spmd(
        nc, in_maps, list(range(NCORES)), trace=trace,
        tmpdir=os.environ.get("KERNEL_TRACE_DIR") or None,
    )
    LAST_RESULTS = res

    out_full = np.broadcast_to(bo, (N, C_Q)).astype(np.float32).copy()
    for c in range(NCORES):
        tok_core = tok_grid[c * GPC : (c + 1) * GPC].reshape(GPC * TPG)
        # out dram: [(GPC//4)*128, 4*C_Q]; [ph][slot p][j, q] -> token
        # (4*ph+j)*TPG + p
        o = (
            np.asarray(res.results[c]["out"])
            .reshape(GPC // 4, TPG, 4, C_Q)
            .transpose(0, 2, 1, 3)
            .reshape(GPC * TPG, C_Q)
        )
        out_full[tok_core] = o + bo
    empty = np.where(sh["counts"] == 0)[0]
    if empty.size:
        out_full[empty] = bo
    return out_full
